# revision 10
# baseline (speedup 1.0000x reference)
"""Trainium2 Bass kernel for nn_CALayer (dynamic local-conv attention layer).

Strategy: pure data parallel over batch B=8 across the 8 NeuronCores.
Each core processes one [64, 128, 128] image.

v3 design notes:
  - Stage A channel-major convs on the PE.  Key 3x3 grouped conv pairs
    taps (di,0)+(di,1) into K=128 matmuls against an [x | x-shifted]
    tile: 6 matmuls/chunk instead of 9.  e2 bias applied via the Act
    cast's bias argument.  relu(e1) lives in a small rolling buffer.
  - Stage C kernel-plane group-broadcast via SBUF->SBUF DMA (one DMA
    per (tap, quarter, half) using a step-0 replication axis), MAC as
    [128, 4096] all-bf16 SBUF DVE ops with both image halves stacked on
    partitions.  Taps with dj in {0,2} read 4B-aligned (DVE 2x mode);
    dj==1 taps pay 1x.
  - Channel attention mean via accum_out of the last MAC op; halves
    folded by a stacked [128, 4] attention lhsT.
"""

import numpy as np

import concourse.bass as bass
import concourse.tile as tile
from concourse import mybir
from concourse.ap import AP
from concourse.bass_utils import run_bass_kernel_spmd

F32 = mybir.dt.float32
BF16 = mybir.dt.bfloat16

H = 128
W = 128
C = 64
PW = 132   # padded width of XX / XV buffers
PH = 130   # padded height of XX
XROWS = 66  # padded rows per half in XV
NPIX = H * W
CHUNK = 512             # pixels per PSUM bank (4 image rows)
NCHUNK = NPIX // CHUNK  # 32
RPC = CHUNK // W        # rows per chunk: 4
HALF = NPIX // 2        # 8192
RQ = 4096               # row-quarter free size for stage C ops
BLK = 4                 # chunks per stage-A block
EPS = 1e-5

AF = mybir.ActivationFunctionType
ALU = mybir.AluOpType


def _split_big_waits(nc, max_waits=1):
    """walrus CTRL codegen accepts only one sem wait per instruction; move
    extra waits onto Drain instructions inserted just before."""
    n_fixed = 0
    for fn in nc.m.functions:
        for bb in fn.blocks:
            insts = bb.instructions
            i = 0
            while i < len(insts):
                inst = insts[i]
                si = inst.sync_info
                if si is not None and si.on_wait and len(si.on_wait) > max_waits:
                    waits = list(si.on_wait)
                    keep = waits[-max_waits:]
                    extra = waits[:-max_waits]
                    new_insts = []
                    for j in range(0, len(extra), max_waits):
                        chunk = extra[j : j + max_waits]
                        d = mybir.InstDrain(
                            name=f"{inst.name}-waitsplit{j}", ins=[], outs=[]
                        )
                        d.engine = inst.engine
                        d.sync_info = mybir.SyncInfo(on_wait=chunk, on_update=[])
                        new_insts.append(d)
                    si.on_wait = keep
                    inst.sync_info = si
                    for k, d in enumerate(new_insts):
                        insts.insert(i + k, d)
                    i += len(new_insts)
                    n_fixed += 1
                i += 1
    return n_fixed


def build_program():
    nc = bass.Bass("TRN2", target_bir_lowering=False, debug=False)

    def din(name, shape, dt=F32):
        return nc.dram_tensor(name, shape, dt, kind="ExternalInput").ap()

    x_in = din("x_shard", [C, H, W])
    wkp = din("wkp", [128, 3, C], BF16)    # paired key lhsT per di: taps (di,0)|(di,1)
    wks = din("wks", [C, 3, C], BF16)      # single key lhsT per di: tap (di,2)
    w1p = din("w1p", [128, 32], BF16)      # e1 lhsT, rows permuted for [x; k]
    w2 = din("w2", [32, 72], BF16)         # e2 lhsT
    be2 = din("b_e2", [72, 1])
    wc1 = din("wc1", [C, C], BF16)         # value-path 1x1 lhsT
    gnw = din("gn_w", [72, 1])
    gnb = din("gn_b", [72, 1])
    g9 = din("g9", [72, 8], BF16)          # group-mean matrix (1/9 entries)
    b72m = din("b72m", [8, 72], BF16)      # group->row broadcast matrix
    wdu1 = din("wdu1s", [128, 4], BF16)    # stacked (half-fold), 1/NPIX folded
    wdu2 = din("wdu2s", [4, 128], BF16)    # duplicated cols
    out_d = nc.dram_tensor("out", [C, H, W], F32, kind="ExternalOutput").ap()

    with tile.TileContext(nc) as tc:
        _build_tile_kernel(
            tc, x_in, wkp, wks, w1p, w2, be2, wc1, gnw, gnb, g9, b72m, wdu1, wdu2, out_d
        )

    _split_big_waits(nc)
    return nc


def _build_tile_kernel(tc, x_in, wkp, wks, w1p, w2, be2, wc1, gnw, gnb, g9, b72m, wdu1, wdu2, out_d):
    nc = tc.nc
    from contextlib import ExitStack

    ctx = ExitStack()
    with ctx:
        big = ctx.enter_context(tc.tile_pool(name="big", bufs=1))
        weights = ctx.enter_context(tc.tile_pool(name="weights", bufs=1))
        psum_k = ctx.enter_context(tc.tile_pool(name="psum_k", bufs=BLK, space="PSUM"))
        psum_1 = ctx.enter_context(tc.tile_pool(name="psum_1", bufs=1, space="PSUM"))
        psum_3 = ctx.enter_context(tc.tile_pool(name="psum_3", bufs=1, space="PSUM"))
        psum_2 = ctx.enter_context(tc.tile_pool(name="psum_2", bufs=2, space="PSUM"))
        small = ctx.enter_context(tc.tile_pool(name="small", bufs=1))
        wbpool = ctx.enter_context(tc.tile_pool(name="wb", bufs=2))
        ppool = ctx.enter_context(tc.tile_pool(name="prod", bufs=1))
        w1pool = ctx.enter_context(tc.tile_pool(name="w1r", bufs=2))
        sgpool = ctx.enter_context(tc.tile_pool(name="sg", bufs=2))
        sopool = ctx.enter_context(tc.tile_pool(name="so", bufs=2))

        # ---- resident big buffers (bf16 activations) ----
        XX = big.tile([128, PH, PW], BF16)   # [x | x-shifted-left-1col], padded
        QK = big.tile([128, NPIX], BF16)     # [x | k], unpadded (e1 rhs)
        W72 = big.tile([72, NPIX], BF16)     # e2 + bias; GN affine in place
        XV = big.tile([128, XROWS, PW], BF16)  # xv halves stacked; img col w at 1+w
        OUT2 = big.tile([128, HALF], BF16)   # local-conv out, halves stacked

        # ---- weights ----
        WKP = weights.tile([128, 3, C], BF16)
        nc.sync.dma_start(WKP[:], wkp[:])
        WKS = weights.tile([C, 3, C], BF16)
        nc.sync.dma_start(WKS[:], wks[:])
        W1P = weights.tile([128, 32], BF16)
        nc.sync.dma_start(W1P[:], w1p[:])
        W2 = weights.tile([32, 72], BF16)
        nc.sync.dma_start(W2[:], w2[:])
        BE2 = weights.tile([72, 1], F32)
        nc.sync.dma_start(BE2[:], be2[:])
        WC1 = weights.tile([C, C], BF16)
        nc.sync.dma_start(WC1[:], wc1[:])
        GNW = weights.tile([72, 1], F32)
        nc.sync.dma_start(GNW[:], gnw[:])
        GNB = weights.tile([72, 1], F32)
        nc.sync.dma_start(GNB[:], gnb[:])
        G9 = weights.tile([72, 8], BF16)
        nc.sync.dma_start(G9[:], g9[:])
        B72 = weights.tile([8, 72], BF16)
        nc.sync.dma_start(B72[:], b72m[:])
        WDU1 = weights.tile([128, 4], BF16)
        nc.sync.dma_start(WDU1[:], wdu1[:])
        WDU2 = weights.tile([4, 128], BF16)
        nc.sync.dma_start(WDU2[:], wdu2[:])

        # ---- zero borders ----
        nc.vector.memset(XX[:, 0:1, :], 0.0)
        nc.vector.memset(XX[:, PH - 1 : PH, :], 0.0)
        # A half: image cols at 2..129 -> pads 0,1 and 130,131
        nc.vector.memset(XX[0:C, 1 : PH - 1, 0:2], 0.0)
        nc.vector.memset(XX[0:C, 1 : PH - 1, 130:132], 0.0)
        # B half: image cols at 1..128 -> pads 0 and 129..131
        nc.vector.memset(XX[C:128, 1 : PH - 1, 0:1], 0.0)
        nc.vector.memset(XX[C:128, 1 : PH - 1, 129:132], 0.0)
        nc.vector.memset(XV[0:C, 0:1, :], 0.0)
        nc.vector.memset(XV[C:128, XROWS - 1 : XROWS, :], 0.0)
        nc.vector.memset(XV[:, :, 0:1], 0.0)
        nc.vector.memset(XV[:, :, 129:130], 0.0)

        # ---- load x (f32), cast into XX-A, derive XX-B and QK-x ----
        IN_ROWS = 8
        for b in range(H // IN_ROWS):
            h0 = b * IN_ROWS
            sg = sgpool.tile([C, IN_ROWS, W], F32, tag="sg")
            nc.sync.dma_start(sg[:], x_in[:, h0 : h0 + IN_ROWS, :])
            # A copy: image col w at 2+w (partitions 0:64)
            nc.vector.tensor_copy(
                XX[0:C, 1 + h0 : 1 + h0 + IN_ROWS, 2 : 2 + W], sg[:]
            )
            # B copy: x shifted left one col -> reading (di,0) AP yields (di,1)
            nc.vector.tensor_copy(
                XX[C:128, 1 + h0 : 1 + h0 + IN_ROWS, 1 : 1 + W], sg[:]
            )
            # QK x half (unpadded)
            qk2 = QK[0:C, h0 * W : (h0 + IN_ROWS) * W].rearrange(
                "p (a b) -> p a b", a=IN_ROWS
            )
            nc.vector.tensor_copy(qk2, sg[:])

        stats = small.tile([72, NCHUNK, 6], F32)
        ys = small.tile([128, 2], F32)

        # ======== stage A: blocked per-chunk convs ========
        for blk in range(NCHUNK // BLK):
            pks = []
            for qi in range(BLK):
                pks.append(
                    psum_k.tile([C, RPC, W], F32, tag="pk", name=f"pk_{blk}_{qi}")
                )
            # key conv: 3 paired (K=128) + 3 single (K=64) matmuls per chunk
            for qi in range(BLK):
                h0 = (blk * BLK + qi) * RPC
                for di in range(3):
                    nc.tensor.matmul(
                        pks[qi][:],
                        WKP[:, di, :],
                        XX[:, h0 + di : h0 + di + RPC, 1 : 1 + W],
                        start=(di == 0),
                        stop=False,
                    )
                for di in range(3):
                    nc.tensor.matmul(
                        pks[qi][:],
                        WKS[:, di, :],
                        XX[0:C, h0 + di : h0 + di + RPC, 3 : 3 + W],
                        start=False,
                        stop=(di == 2),
                    )
            w1r = w1pool.tile([32, BLK * CHUNK], BF16, tag="w1r")
            for qi in range(BLK):
                q = blk * BLK + qi
                h0 = q * RPC
                qs = slice(q * CHUNK, (q + 1) * CHUNK)
                rs = slice(qi * CHUNK, (qi + 1) * CHUNK)
                half = 0 if q < NCHUNK // 2 else 1
                plo, phi = (0, C) if half == 0 else (C, 128)

                # k -> QK[64:128]
                kdst = QK[C:128, qs].rearrange("p (a b) -> p a b", a=RPC)
                nc.scalar.activation(kdst, pks[qi][:], AF.Relu)

                # e1: [128 -> 32] + relu into rolling buffer
                p1 = psum_1.tile([32, CHUNK], F32, tag="p1")
                nc.tensor.matmul(p1[:], W1P[:], QK[:, qs], start=True, stop=True)
                nc.scalar.activation(w1r[:, rs], p1[:], AF.Relu)

                # e2: [32 -> 72] + bias via Act
                p3 = psum_3.tile([72, CHUNK], F32, tag="p3")
                nc.tensor.matmul(p3[:], W2[:], w1r[:, rs], start=True, stop=True)
                nc.scalar.activation(W72[:, qs], p3[:], AF.Identity, bias=BE2[:])
                nc.vector.bn_stats(out=stats[:, q, :], in_=W72[:, qs])

                # c1 value path -> XV (half-stacked, img col w at 1+w)
                p2 = psum_2.tile([128, CHUNK], F32, tag="p2")
                nc.tensor.matmul(
                    p2[plo:phi], WC1[:],
                    XX[0:C, 1 + h0 : 1 + h0 + RPC, 2 : 2 + W],
                    start=True, stop=True,
                )
                hr0 = h0 - half * (H // 2)
                p2v = p2[plo:phi].rearrange("p (a b) -> p a b", a=RPC)
                nc.scalar.activation(
                    XV[plo:phi, 1 + hr0 : 1 + hr0 + RPC, 1 : 1 + W], p2v, AF.Identity
                )

                # half-boundary duplicated rows
                if q == NCHUNK // 2 - 1:
                    pbt = psum_2.tile([128, CHUNK], F32, tag="p2")
                    pb = pbt[:, 0:W]
                    nc.tensor.matmul(
                        pb[C:128], WC1[:],
                        XX[0:C, 1 + 63 : 1 + 64, 2 : 2 + W],
                        start=True, stop=True,
                    )
                    nc.scalar.activation(XV[C:128, 0:1, 1 : 1 + W], pb[C:128], AF.Identity)
                if q == NCHUNK // 2:
                    pbt = psum_2.tile([128, CHUNK], F32, tag="p2")
                    pb = pbt[:, 0:W]
                    nc.tensor.matmul(
                        pb[0:C], WC1[:],
                        XX[0:C, 1 + 64 : 1 + 65, 2 : 2 + W],
                        start=True, stop=True,
                    )
                    nc.scalar.activation(
                        XV[0:C, XROWS - 1 : XROWS, 1 : 1 + W], pb[0:C], AF.Identity
                    )

        # ======== stage B: GroupNorm scale/bias ========
        mv = small.tile([72, 2], F32)
        nc.vector.bn_aggr(out=mv[:], in_=stats[:])
        rowq = small.tile([72, 1], F32)
        nc.vector.tensor_mul(rowq[:], mv[:, 0:1], mv[:, 0:1])
        nc.vector.tensor_add(rowq[:], rowq[:], mv[:, 1:2])
        pack = small.tile([72, 2], BF16)
        nc.vector.tensor_copy(pack[:, 0:1], mv[:, 0:1])
        nc.vector.tensor_copy(pack[:, 1:2], rowq[:])
        pgt = psum_2.tile([128, CHUNK], F32, tag="p2")
        pg = pgt[0:8, 0:2]
        nc.tensor.matmul(pg, G9[:], pack[:], start=True, stop=True)
        gm = small.tile([8, 2], F32)  # (m_g, E[w^2]_g)
        nc.vector.tensor_copy(gm[:], pg)
        msq = small.tile([8, 1], F32)
        nc.vector.tensor_mul(msq[:], gm[:, 0:1], gm[:, 0:1])
        v8 = small.tile([8, 1], F32)
        nc.vector.tensor_tensor(out=v8[:], in0=gm[:, 1:2], in1=msq[:], op=ALU.subtract)
        eps8 = small.tile([8, 1], F32)
        nc.vector.memset(eps8[:], EPS)
        sd8 = small.tile([8, 1], F32)
        nc.scalar.activation(sd8[:], v8[:], AF.Sqrt, bias=eps8[:])
        rstd8 = small.tile([8, 2], F32)  # col 0: rstd, col 1: mean
        nc.vector.reciprocal(rstd8[:, 0:1], sd8[:])
        nc.vector.tensor_copy(rstd8[:, 1:2], gm[:, 0:1])
        rstd8b = small.tile([8, 2], BF16)
        nc.vector.tensor_copy(rstd8b[:], rstd8[:])
        p72t = psum_2.tile([128, CHUNK], F32, tag="p2")
        p72 = p72t[0:72, 0:2]
        nc.tensor.matmul(p72, B72[:], rstd8b[:], start=True, stop=True)
        rs72 = small.tile([72, 2], F32)
        nc.vector.tensor_copy(rs72[:], p72)
        a72 = small.tile([72, 1], F32)
        nc.vector.tensor_mul(a72[:], rs72[:, 0:1], GNW[:])
        b72 = small.tile([72, 1], F32)
        nc.vector.tensor_mul(b72[:], rs72[:, 1:2], a72[:])
        nc.vector.tensor_tensor(out=b72[:], in0=GNB[:], in1=b72[:], op=ALU.subtract)
        # apply GN affine in place, ordered so stage C quarters unblock early
        for seg in range(4):
            ss = slice(seg * RQ, (seg + 1) * RQ)
            nc.scalar.activation(
                W72[:, ss], W72[:, ss], AF.Identity, bias=b72[:], scale=a72[:]
            )

        # ======== stage C: per-pixel local conv ========
        # wb[64*half + g*8 + c8, px] = W72[g*9 + t, half*HALF + px]
        taps = [(di, dj) for di in range(3) for dj in range(3)]
        for t, (di, dj) in enumerate(taps):
            wb = wbpool.tile([128, HALF], BF16, tag="wb")
            for half in range(2):
                src = W72[t : t + 64 : 9, half * HALF : (half + 1) * HALF]
                rep = AP(src.tensor, src.offset,
                         [src.ap[0], [0, 8], src.ap[1]])
                eng = nc.sync if half == 0 else nc.scalar
                eng.dma_start(wb[64 * half : 64 * half + 64, :], rep)
            for rq in range(HALF // RQ):
                rqs = slice(rq * RQ, (rq + 1) * RQ)
                hh0 = rq * (RQ // W)  # 0 or 32 (half-relative row base)
                wbv = wb[:, rqs].rearrange("p (a b) -> p a b", a=RQ // W)
                xs = XV[:, hh0 + di : hh0 + di + RQ // W, dj : dj + W]
                outc = OUT2[:, rqs].rearrange("p (a b) -> p a b", a=RQ // W)
                if t == 0:
                    nc.vector.tensor_mul(outc, xs, wbv)
                else:
                    p = ppool.tile([128, RQ], BF16, tag="p", name=f"p_{t}_{rq}")
                    pv = p[:].rearrange("p (a b) -> p a b", a=RQ // W)
                    nc.vector.tensor_mul(pv, xs, wbv)
                    if t < 8:
                        nc.vector.tensor_add(outc, outc, pv)
                    else:
                        nc.vector.scalar_tensor_tensor(
                            out=outc,
                            in0=outc,
                            scalar=0.0,
                            in1=pv,
                            op0=ALU.add,
                            op1=ALU.add,
                            accum_out=ys[:, rq : rq + 1],
                        )

        # ======== stage D: channel attention ========
        ysum = small.tile([128, 1], F32)
        nc.vector.tensor_reduce(ysum[:], ys[:], axis=mybir.AxisListType.X, op=ALU.add)
        ysb = small.tile([128, 1], BF16)
        nc.vector.tensor_copy(ysb[:], ysum[:])
        pa1t = psum_2.tile([128, CHUNK], F32, tag="p2")
        pa1 = pa1t[0:4, 0:1]
        nc.tensor.matmul(pa1, WDU1[:], ysb[:], start=True, stop=True)
        y1 = small.tile([4, 1], BF16)
        nc.scalar.activation(y1[:], pa1, AF.Relu)
        pa2t = psum_2.tile([128, CHUNK], F32, tag="p2")
        pa2 = pa2t[:, 0:1]
        nc.tensor.matmul(pa2, WDU2[:], y1[:], start=True, stop=True)
        yatt = small.tile([128, 1], F32)
        nc.scalar.activation(yatt[:], pa2, AF.Sigmoid)

        # ======== final scale + store ========
        out_flat = out_d.rearrange("c h w -> c (h w)")
        OB = 1024
        for ob in range(HALF // OB):
            obs = slice(ob * OB, (ob + 1) * OB)
            so = sopool.tile([128, OB], F32, tag="so")
            nc.scalar.activation(so[:], OUT2[:, obs], AF.Identity, scale=yatt[:])
            nc.sync.dma_start(out_flat[:, ob * OB : ob * OB + OB], so[0:C, :])
            nc.sync.dma_start(
                out_flat[:, HALF + ob * OB : HALF + ob * OB + OB], so[C:128, :]
            )


def prep_weights(w_key, w_e1, w_e2, b_e2, gn_w, gn_b, w_c1, w_du1, w_du2):
    import ml_dtypes

    bf = ml_dtypes.bfloat16
    wk_taps = np.zeros((C, 9, C), np.float32)
    for di in range(3):
        for dj in range(3):
            t = di * 3 + dj
            for o in range(C):
                g = o // 8
                for j in range(8):
                    wk_taps[g * 8 + j, t, o] = w_key[o, j, di, dj]
    # paired lhsT: rows 0:64 tap (di,0) (x copy A), rows 64:128 tap (di,1) (x copy B)
    wkp = np.zeros((128, 3, C), np.float32)
    wks = np.zeros((C, 3, C), np.float32)
    for di in range(3):
        wkp[0:C, di, :] = wk_taps[:, 3 * di + 0, :]
        wkp[C:128, di, :] = wk_taps[:, 3 * di + 1, :]
        wks[:, di, :] = wk_taps[:, 3 * di + 2, :]

    w1p = np.zeros((128, 32), np.float32)
    for r in range(128):
        if r < 64:
            qki = 2 * r
        else:
            qki = 2 * (r - 64) + 1
        if qki < 64:
            w1p[r, 0:16] = w_e1[0:16, qki, 0, 0]
        else:
            w1p[r, 16:32] = w_e1[16:32, qki - 64, 0, 0]

    w2 = np.zeros((32, 72), np.float32)
    for j in range(32):
        if j < 16:
            w2[j, 0:36] = w_e2[0:36, j, 0, 0]
        else:
            w2[j, 36:72] = w_e2[36:72, j - 16, 0, 0]

    wc1 = np.zeros((C, C), np.float32)
    for o in range(C):
        if o < 32:
            wc1[0:32, o] = w_c1[o, :, 0, 0]
        else:
            wc1[32:64, o] = w_c1[o, :, 0, 0]

    wdu1 = (w_du1[:, :, 0, 0].T / float(NPIX)).astype(np.float32)  # [64, 4]
    wdu1s = np.concatenate([wdu1, wdu1], axis=0)  # [128, 4] (half fold)
    wdu2 = w_du2[:, :, 0, 0].T.astype(np.float32)  # [4, 64]
    wdu2s = np.concatenate([wdu2, wdu2], axis=1)  # [4, 128]

    g9 = np.zeros((72, 8), np.float32)
    for r in range(72):
        g9[r, r // 9] = 1.0 / 9.0
    b72m = np.zeros((8, 72), np.float32)
    for r in range(72):
        b72m[r // 9, r] = 1.0
    return {
        "g9": g9.astype(bf),
        "b72m": b72m.astype(bf),
        "wkp": wkp.astype(bf),
        "wks": wks.astype(bf),
        "w1p": w1p.astype(bf),
        "w2": w2.astype(bf),
        "b_e2": b_e2.reshape(72, 1).astype(np.float32),
        "wc1": wc1.astype(bf),
        "gn_w": gn_w.reshape(72, 1).astype(np.float32),
        "gn_b": gn_b.reshape(72, 1).astype(np.float32),
        "wdu1s": wdu1s.astype(bf),
        "wdu2s": wdu2s.astype(bf),
    }


_PROGRAM_CACHE = {}


def _get_program():
    if "nc" not in _PROGRAM_CACHE:
        _PROGRAM_CACHE["nc"] = build_program()
    return _PROGRAM_CACHE["nc"]


def run_on_cores(inputs, trace=False):
    nc = _get_program()
    x = np.asarray(inputs["x"], np.float32)
    wmaps = prep_weights(
        np.asarray(inputs["w_key"], np.float32),
        np.asarray(inputs["w_e1"], np.float32),
        np.asarray(inputs["w_e2"], np.float32),
        np.asarray(inputs["b_e2"], np.float32),
        np.asarray(inputs["gn_w"], np.float32),
        np.asarray(inputs["gn_b"], np.float32),
        np.asarray(inputs["w_c1"], np.float32),
        np.asarray(inputs["w_du1"], np.float32),
        np.asarray(inputs["w_du2"], np.float32),
    )
    in_maps = []
    for b in range(8):
        m = {"x_shard": np.ascontiguousarray(x[b])}
        m.update(wmaps)
        in_maps.append(m)
    res = run_bass_kernel_spmd(nc, in_maps, core_ids=list(range(8)), trace=trace)
    out = np.stack([res.results[b]["out"] for b in range(8)], axis=0)
    return out, res


def kernel(**inputs) -> np.ndarray:
    out, _ = run_on_cores(inputs, trace=False)
    return out.astype(np.float32)
